# revision 4
# baseline (speedup 1.0000x reference)
"""Bounding-box discipline penalty kernel for Trainium2 (8 NeuronCores).

Reference computation:
    pred_mask = max_c(prediction_probs) > 0.3   [B, H, W]
    true_mask = max_c(expected_onehot)  > 0.5   [B, H, W]
    per-sample bboxes from the masks -> area/center penalties -> scalar mean.

Strategy (pure data parallel, B=16 over 8 cores => 2 samples/core):
  * Device: stream both tensors' shards through SBUF and compute the
    per-pixel channel max, laid out as pixmax[partition=128, 512] per
    (tensor, sample). That is the entire memory-bound part (reads 128 MiB
    per core at HBM line rate; DVE reduce overlaps the DMA).
  * Host: fold the tiny [4, 128, 512] per-core results into per-sample
    row/col maxima (exact max operations, order-independent), then do the
    O(B) bbox + penalty math exactly as the reference does.

Self-contained: hardcodes shapes from the problem spec.
"""

import numpy as np

THRESHOLD = 0.3
PENALTY_WEIGHT = 0.05

B, H, W, C = 16, 256, 256, 128
N_CORES = 8
SPC = B // N_CORES            # samples per core = 2
NST = 2 * SPC                 # sample-tensor streams per core = 4
PIX = H * W                   # 65536 pixels per sample
NPART = 128
PPP = PIX // NPART            # 512 pixels per partition
EPP = PPP * C                 # 65536 f32 elems per partition per sample
NT = 4                        # tiles per sample-tensor
F = EPP // NT                 # 16384 elems/partition per DMA (8 MiB tiles)
PXT = F // C                  # 128 pixels per partition per tile
NLOADS = NST * NT             # 16 big loads per core

_cache = {}


def _build_nc():
    import concourse.bass as bass
    import concourse.mybir as mybir

    f32 = mybir.dt.float32
    nc = bass.Bass()
    pred = nc.dram_tensor("pred", [SPC, NPART, EPP], f32, kind="ExternalInput")
    tru = nc.dram_tensor("tru", [SPC, NPART, EPP], f32, kind="ExternalInput")
    # pixmax per sample-tensor: [st, partition, pixel-in-partition]
    outp = nc.dram_tensor("outp", [NST, NPART, PPP], f32, kind="ExternalOutput")

    srcs = [(pred, 0), (pred, 1), (tru, 0), (tru, 1)]

    with (
        nc.sbuf_tensor([NPART, F], f32) as buf0,
        nc.sbuf_tensor([NPART, F], f32) as buf1,
        nc.sbuf_tensor([NPART, PPP], f32) as pm0,
        nc.sbuf_tensor([NPART, PPP], f32) as pm1,
        nc.semaphore("ls0") as ls0,
        nc.semaphore("ls1") as ls1,
        nc.semaphore("bufree") as bufree,
        nc.semaphore("os0") as os0,
        nc.semaphore("os1") as os1,
        nc.Block() as block,
    ):
        lsems = [ls0, ls1]
        osems = [os0, os1]
        buf = [buf0, buf1]
        pm = [pm0, pm1]

        @block.sync
        def _(sync):
            for k in range(NLOADS):
                st, i = divmod(k, NT)
                src, s = srcs[st]
                if k >= 2:
                    # buffer k%2 reused from load k-2: wait for its reduce
                    sync.wait_ge(bufree, k - 1)
                sync.dma_start(
                    out=buf[k % 2][:], in_=src[s, :, i * F : (i + 1) * F]
                ).then_inc(lsems[k % 2], 16)

        @block.vector
        def _(vector):
            for k in range(NLOADS):
                st, i = divmod(k, NT)
                if i == 0 and st >= 2:
                    # WAR: pm[st%2] may still be DMA-ing out for st-2
                    vector.wait_ge(osems[st % 2], 16 * (st // 2))
                vector.wait_ge(lsems[k % 2], 16 * (k // 2 + 1))
                vector.reduce_max(
                    out=pm[st % 2][:, i * PXT : (i + 1) * PXT],
                    in_=buf[k % 2][:].rearrange("p (a c) -> p a c", c=C),
                    axis=mybir.AxisListType.X,
                ).then_inc(bufree, 1)

        @block.scalar
        def _(scalar):
            for st in range(NST):
                scalar.wait_ge(bufree, NT * (st + 1))
                scalar.dma_start(out=outp[st], in_=pm[st % 2][:]).then_inc(
                    osems[st % 2], 16
                )
            scalar.wait_ge(osems[0], 16 * (NST - NST // 2))
            scalar.wait_ge(osems[1], 16 * (NST // 2))

    return nc


def _run_device(pred_np, true_np, trace=False):
    from concourse.bass_utils import run_bass_kernel_spmd

    if "nc" not in _cache:
        _cache["nc"] = _build_nc()
    nc = _cache["nc"]

    # [B, H, W, C] -> per-core shards [SPC, 128, EPP]
    pred_sh = pred_np.reshape(N_CORES, SPC, NPART, EPP)
    true_sh = true_np.reshape(N_CORES, SPC, NPART, EPP)
    in_maps = [
        {"pred": pred_sh[i], "tru": true_sh[i]} for i in range(N_CORES)
    ]
    res = run_bass_kernel_spmd(
        nc, in_maps, core_ids=list(range(N_CORES)), trace=trace
    )
    # [N_CORES, NST, 128, PPP]
    pms = np.stack([res.results[i]["outp"] for i in range(N_CORES)])
    return pms, res


def _bbox_from_maxes(rowv, colv, thresh):
    """rowv [B,H], colv [B,W] float32 maxima -> bbox coords, matching _bbox."""
    row_any = rowv > thresh
    col_any = colv > thresh
    ys = np.arange(H, dtype=np.float32)
    xs = np.arange(W, dtype=np.float32)
    y_min = np.where(row_any, ys, np.float32(H)).min(axis=1)
    y_max = np.where(row_any, ys, np.float32(-1)).max(axis=1)
    x_min = np.where(col_any, xs, np.float32(W)).min(axis=1)
    x_max = np.where(col_any, xs, np.float32(-1)).max(axis=1)
    empty = ~row_any.any(axis=1)
    f32 = np.float32
    y_min = np.where(empty, f32(0.0), y_min).astype(np.float32)
    x_min = np.where(empty, f32(0.0), x_min).astype(np.float32)
    y_max = np.where(empty, f32(1.0), y_max).astype(np.float32)
    x_max = np.where(empty, f32(1.0), x_max).astype(np.float32)
    return y_min, x_min, y_max, x_max


def _penalty_from_pms(pms):
    """pms [N_CORES, NST, 128, PPP] -> scalar penalty (float32)."""
    # pms[c, st] covers sample 2c + (st % SPC); st//SPC==0 -> pred, ==1 -> true
    pm4 = pms.reshape(N_CORES, 2, SPC, NPART, 2, W)  # [c, tensor, s, p, r, w]
    pm4 = pm4.transpose(1, 0, 2, 3, 4, 5).reshape(2, B, NPART, 2, W)
    rowv = pm4.max(axis=4)            # [2, B, 128, 2] -> rows 2p+r
    rowv = rowv.reshape(2, B, H)
    colv = pm4.max(axis=(2, 3))       # [2, B, W]

    p = _bbox_from_maxes(rowv[0], colv[0], np.float32(THRESHOLD))
    t = _bbox_from_maxes(rowv[1], colv[1], np.float32(0.5))
    py_min, px_min, py_max, px_max = p
    ty_min, tx_min, ty_max, tx_max = t

    one = np.float32(1.0)
    pred_area = (py_max - py_min + one) * (px_max - px_min + one)
    true_area = (ty_max - ty_min + one) * (tx_max - tx_min + one)
    area_penalty = np.maximum(pred_area - true_area, np.float32(0.0)) / (
        true_area + one
    )
    two = np.float32(2.0)
    dy = (py_min + py_max) / two - (ty_min + ty_max) / two
    dx = (px_min + px_max) / two - (tx_min + tx_max) / two
    center_offset = np.sqrt(dy * dy + dx * dx).astype(np.float32) / np.float32(
        20.0
    )
    penalties = area_penalty + center_offset
    return np.float32(PENALTY_WEIGHT) * penalties.mean(dtype=np.float32)


def _run(prediction_probs, expected_onehot, trace=False):
    pred_np = np.ascontiguousarray(
        np.asarray(prediction_probs, dtype=np.float32)
    )
    true_np = np.ascontiguousarray(
        np.asarray(expected_onehot, dtype=np.float32)
    )
    assert pred_np.shape == (B, H, W, C), pred_np.shape
    assert true_np.shape == (B, H, W, C), true_np.shape
    pms, res = _run_device(pred_np, true_np, trace=trace)
    val = _penalty_from_pms(pms)
    return np.asarray(val, dtype=np.float32), res


def kernel(prediction_probs, expected_onehot):
    out, _ = _run(prediction_probs, expected_onehot, trace=False)
    return out


# revision 6
# speedup vs baseline: 1.0597x; 1.0597x over previous
"""Bounding-box discipline penalty kernel for Trainium2 (8 NeuronCores).

Reference computation:
    pred_mask = max_c(prediction_probs) > 0.3   [B, H, W]
    true_mask = max_c(expected_onehot)  > 0.5   [B, H, W]
    per-sample bboxes from the masks -> area/center penalties -> scalar mean.

Strategy (pure data parallel, B=16 over 8 cores => 2 samples/core):
  * Device: stream both tensors' shards through SBUF and compute the
    per-pixel channel max, laid out as pixmax[partition=128, 512] per
    (tensor, sample). That is the entire memory-bound part (reads 128 MiB
    per core at HBM line rate; DVE reduce overlaps the DMA).
  * Host: fold the tiny [4, 128, 512] per-core results into per-sample
    row/col maxima (exact max operations, order-independent), then do the
    O(B) bbox + penalty math exactly as the reference does.

Self-contained: hardcodes shapes from the problem spec.
"""

import numpy as np

THRESHOLD = 0.3
PENALTY_WEIGHT = 0.05

B, H, W, C = 16, 256, 256, 128
N_CORES = 8
SPC = B // N_CORES            # samples per core = 2
NST = 2 * SPC                 # sample-tensor streams per core = 4
PIX = H * W                   # 65536 pixels per sample
NPART = 128
PPP = PIX // NPART            # 512 pixels per partition
EPP = PPP * C                 # 65536 f32 elems per partition per sample
NT = 4                        # tiles per sample-tensor
F = EPP // NT                 # 16384 elems/partition per DMA (8 MiB tiles)
PXT = F // C                  # 128 pixels per partition per tile
NLOADS = NST * NT             # 16 big loads per core

_cache = {}


def _chunk_schedule():
    """Per-sample-tensor chunk sizes (f32 elems per partition).

    Uniform 8 MiB chunks, except the last sample-tensor tapers off so the
    final DVE reduce (which is serial after the last DMA lands) is short.
    """
    per_st = []
    for st in range(NST):
        if st < NST - 1:
            per_st.append([F] * NT)
        else:
            per_st.append([F] * (NT - 1) + [F // 2, F // 4, F // 8, F // 16, F // 16])
    for sizes in per_st:
        assert sum(sizes) == EPP
    # flat list of (st, elem offset, size, chunk-idx-in-st, is-last-of-st)
    loads = []
    for st, sizes in enumerate(per_st):
        off = 0
        for j, sz in enumerate(sizes):
            loads.append((st, off, sz, j == len(sizes) - 1))
            off += sz
    return per_st, loads


def _build_nc():
    import concourse.bass as bass
    import concourse.mybir as mybir

    f32 = mybir.dt.float32
    nc = bass.Bass()
    pred = nc.dram_tensor("pred", [SPC, NPART, EPP], f32, kind="ExternalInput")
    tru = nc.dram_tensor("tru", [SPC, NPART, EPP], f32, kind="ExternalInput")
    # pixmax per sample-tensor: [st, partition, pixel-in-partition]
    outp = nc.dram_tensor("outp", [NST, NPART, PPP], f32, kind="ExternalOutput")

    srcs = [(pred, 0), (pred, 1), (tru, 0), (tru, 1)]
    per_st, loads = _chunk_schedule()
    nloads = len(loads)
    # reduces completed (== bufree count) after finishing each st
    done_after_st = []
    acc = 0
    for sizes in per_st:
        acc += len(sizes)
        done_after_st.append(acc)

    with (
        nc.sbuf_tensor([NPART, F], f32) as buf0,
        nc.sbuf_tensor([NPART, F], f32) as buf1,
        nc.sbuf_tensor([NPART, PPP], f32) as pm0,
        nc.sbuf_tensor([NPART, PPP], f32) as pm1,
        nc.semaphore("ls0") as ls0,
        nc.semaphore("ls1") as ls1,
        nc.semaphore("bufree") as bufree,
        nc.semaphore("os0") as os0,
        nc.semaphore("os1") as os1,
        nc.Block() as block,
    ):
        lsems = [ls0, ls1]
        osems = [os0, os1]
        buf = [buf0, buf1]
        pm = [pm0, pm1]
        # out-DMA count (x16) per parity, accumulated in program order
        out_counts = [0, 0]

        @block.sync
        def _(sync):
            for k, (st, off, sz, _last) in enumerate(loads):
                src, s = srcs[st]
                if k >= 2:
                    # buffer k%2 reused from load k-2: wait for its reduce
                    sync.wait_ge(bufree, k - 1)
                sync.dma_start(
                    out=buf[k % 2][:, :sz], in_=src[s, :, off : off + sz]
                ).then_inc(lsems[k % 2], 16)

        @block.vector
        def _(vector):
            for k, (st, off, sz, _last) in enumerate(loads):
                if off == 0 and st >= 2:
                    # WAR: pm[st%2] may still be DMA-ing out for st-2
                    vector.wait_ge(osems[st % 2], 16 * (st // 2))
                vector.wait_ge(lsems[k % 2], 16 * (k // 2 + 1))
                vector.reduce_max(
                    out=pm[st % 2][:, off // C : (off + sz) // C],
                    in_=buf[k % 2][:, :sz].rearrange("p (a c) -> p a c", c=C),
                    axis=mybir.AxisListType.X,
                ).then_inc(bufree, 1)

        @block.scalar
        def _(scalar):
            for st in range(NST):
                sizes = per_st[st]
                par = st % 2
                if len(sizes) == NT:
                    scalar.wait_ge(bufree, done_after_st[st])
                    scalar.dma_start(out=outp[st], in_=pm[par][:]).then_inc(
                        osems[par], 16
                    )
                    out_counts[par] += 1
                else:
                    # tapered st: flush the pixels covered by the first
                    # NT-1 chunks early, then the small remainder at the end
                    head_px = sum(sizes[: NT - 1]) // C
                    base = done_after_st[st - 1] if st else 0
                    scalar.wait_ge(bufree, base + NT - 1)
                    scalar.dma_start(
                        out=outp[st, :, :head_px], in_=pm[par][:, :head_px]
                    ).then_inc(osems[par], 16)
                    out_counts[par] += 1
                    scalar.wait_ge(bufree, done_after_st[st])
                    scalar.dma_start(
                        out=outp[st, :, head_px:], in_=pm[par][:, head_px:]
                    ).then_inc(osems[par], 16)
                    out_counts[par] += 1
            scalar.wait_ge(osems[0], 16 * out_counts[0])
            scalar.wait_ge(osems[1], 16 * out_counts[1])

    return nc


def _run_device(pred_np, true_np, trace=False):
    from concourse.bass_utils import run_bass_kernel_spmd

    if "nc" not in _cache:
        _cache["nc"] = _build_nc()
    nc = _cache["nc"]

    # [B, H, W, C] -> per-core shards [SPC, 128, EPP]
    pred_sh = pred_np.reshape(N_CORES, SPC, NPART, EPP)
    true_sh = true_np.reshape(N_CORES, SPC, NPART, EPP)
    in_maps = [
        {"pred": pred_sh[i], "tru": true_sh[i]} for i in range(N_CORES)
    ]
    res = run_bass_kernel_spmd(
        nc, in_maps, core_ids=list(range(N_CORES)), trace=trace
    )
    # [N_CORES, NST, 128, PPP]
    pms = np.stack([res.results[i]["outp"] for i in range(N_CORES)])
    return pms, res


def _bbox_from_maxes(rowv, colv, thresh):
    """rowv [B,H], colv [B,W] float32 maxima -> bbox coords, matching _bbox."""
    row_any = rowv > thresh
    col_any = colv > thresh
    ys = np.arange(H, dtype=np.float32)
    xs = np.arange(W, dtype=np.float32)
    y_min = np.where(row_any, ys, np.float32(H)).min(axis=1)
    y_max = np.where(row_any, ys, np.float32(-1)).max(axis=1)
    x_min = np.where(col_any, xs, np.float32(W)).min(axis=1)
    x_max = np.where(col_any, xs, np.float32(-1)).max(axis=1)
    empty = ~row_any.any(axis=1)
    f32 = np.float32
    y_min = np.where(empty, f32(0.0), y_min).astype(np.float32)
    x_min = np.where(empty, f32(0.0), x_min).astype(np.float32)
    y_max = np.where(empty, f32(1.0), y_max).astype(np.float32)
    x_max = np.where(empty, f32(1.0), x_max).astype(np.float32)
    return y_min, x_min, y_max, x_max


def _penalty_from_pms(pms):
    """pms [N_CORES, NST, 128, PPP] -> scalar penalty (float32)."""
    # pms[c, st] covers sample 2c + (st % SPC); st//SPC==0 -> pred, ==1 -> true
    pm4 = pms.reshape(N_CORES, 2, SPC, NPART, 2, W)  # [c, tensor, s, p, r, w]
    pm4 = pm4.transpose(1, 0, 2, 3, 4, 5).reshape(2, B, NPART, 2, W)
    rowv = pm4.max(axis=4)            # [2, B, 128, 2] -> rows 2p+r
    rowv = rowv.reshape(2, B, H)
    colv = pm4.max(axis=(2, 3))       # [2, B, W]

    p = _bbox_from_maxes(rowv[0], colv[0], np.float32(THRESHOLD))
    t = _bbox_from_maxes(rowv[1], colv[1], np.float32(0.5))
    py_min, px_min, py_max, px_max = p
    ty_min, tx_min, ty_max, tx_max = t

    one = np.float32(1.0)
    pred_area = (py_max - py_min + one) * (px_max - px_min + one)
    true_area = (ty_max - ty_min + one) * (tx_max - tx_min + one)
    area_penalty = np.maximum(pred_area - true_area, np.float32(0.0)) / (
        true_area + one
    )
    two = np.float32(2.0)
    dy = (py_min + py_max) / two - (ty_min + ty_max) / two
    dx = (px_min + px_max) / two - (tx_min + tx_max) / two
    center_offset = np.sqrt(dy * dy + dx * dx).astype(np.float32) / np.float32(
        20.0
    )
    penalties = area_penalty + center_offset
    return np.float32(PENALTY_WEIGHT) * penalties.mean(dtype=np.float32)


def _run(prediction_probs, expected_onehot, trace=False):
    pred_np = np.ascontiguousarray(
        np.asarray(prediction_probs, dtype=np.float32)
    )
    true_np = np.ascontiguousarray(
        np.asarray(expected_onehot, dtype=np.float32)
    )
    assert pred_np.shape == (B, H, W, C), pred_np.shape
    assert true_np.shape == (B, H, W, C), true_np.shape
    pms, res = _run_device(pred_np, true_np, trace=trace)
    val = _penalty_from_pms(pms)
    return np.asarray(val, dtype=np.float32), res


def kernel(prediction_probs, expected_onehot):
    out, _ = _run(prediction_probs, expected_onehot, trace=False)
    return out
